# revision 1
# baseline (speedup 1.0000x reference)
"""Bass/Tile GATv2 kernel for TRN2, dst-sharded across 8 cores.

Two GATv2 layers (+linear residual), each run as one SPMD program over 8
NeuronCores. Host side: nodes are bin-packed into (core, tile, slot) so every
128-dst tile has nearly equal edge count (NB blocks of 128 edges); one-hot
edge->dst matrices (oh) and their transposes (ohT) are precomputed on host and
streamed to the device; source features are compacted per subphase so int16
gather indices suffice.

Device side per dst tile (all feature tensors channel-major: col = c*H + h):
  - DMA oh/ohT stream + dma_gather of xl rows (from a DRAM table built in
    phase 1 as x_compact @ wl)
  - xr = xtloc @ wr (PE) per tile
  - pass 1 per 128-edge block: s = ohT^T@xr + I@xlg (PE, PSUM), lr =
    prelu(s) (scalar), lr *= att (DVE 2x), fold+reduce -> e[:,8] (DVE)
  - one exp over all blocks' e (scalar)
  - pass 2 per block: xlw = xlg * exp-bcast (DVE 2x, ch-major), U += oh^T@xlw,
    D += oh^T@ex (PE)
  - epilogue: out = mean_h(U/D) + bias [+res / relu]
"""
from contextlib import ExitStack
from dataclasses import dataclass

import numpy as np

import concourse.bass as bass
import concourse.tile as tile
from concourse import bacc, mybir
from concourse.masks import make_identity

F32 = mybir.dt.float32
BF16 = mybir.dt.bfloat16
I16 = mybir.dt.int16
AF = mybir.ActivationFunctionType
ALU = mybir.AluOpType
AX = mybir.AxisListType
P = 128


@dataclass
class LayerCfg:
    Fin: int            # contraction dim of input features (128 L1, 64 L2)
    H: int              # heads
    CH: int             # per-head channels (64 L1, 32 L2)
    NT: int             # dst tiles per core
    NB: int             # edge blocks per tile (uniform across cores/tiles)
    TB: tuple           # tile-group boundaries, len NSUB+1
    ROWPADS: tuple      # compact-table rows per subphase (128-mult, uniform)
    relu: bool = False
    calc_residual: bool = False
    add_residual: bool = False
    RESC: int = 32
    neg_slope: float = 0.2
    chunk: int = 2048   # xTc column chunk for phase 1
    KB: int = 0         # blocks per tile scored via PE (transposed path)

    @property
    def HC(self):
        return self.H * self.CH

    @property
    def NBLK(self):
        return self.HC // P


def build_layer(nc: bacc.Bacc, cfg: LayerCfg):
    HC, H, CH, Fin, NT, NB = cfg.HC, cfg.H, cfg.CH, cfg.Fin, cfg.NT, cfg.NB
    NLOC = NT * P
    NIDX = NB * P
    IDXW = NIDX // 16
    OHW = 2 * NB * P          # oh+ohT columns per tile

    dt = nc.dram_tensor
    NSUB = len(cfg.ROWPADS)
    xTc_ts = [dt(f"xTc{i}", (Fin, cfg.ROWPADS[i]), BF16, kind="ExternalInput")
              for i in range(NSUB)]
    xTloc_t = dt("xTloc", (Fin, NLOC), BF16, kind="ExternalInput")
    if cfg.KB:
        # row-major compact inputs (feature dim zero-padded to 128) for the
        # transposed B-path gathers
        xcm_ts = [dt(f"xcm{i}", (cfg.ROWPADS[i], P), BF16,
                     kind="ExternalInput") for i in range(NSUB)]
    wl_t = dt("wl", (P, HC), BF16, kind="ExternalInput")
    wr_t = dt("wr", (Fin, HC), BF16, kind="ExternalInput")
    attb_t = dt("attb", (P, 2 * HC), BF16, kind="ExternalInput")
    if cfg.KB:
        attT_t = dt("attT", (P, cfg.NBLK * H), BF16, kind="ExternalInput")
    bbc_t = dt("bbc", (P, CH), BF16, kind="ExternalInput")
    idx16_t = dt("idx16", (P, NT * IDXW), I16, kind="ExternalInput")
    ohall_t = dt("ohall", (P, NT * OHW), BF16, kind="ExternalInput")
    if cfg.calc_residual:
        linw_t = dt("linw", (Fin, cfg.RESC), BF16, kind="ExternalInput")
        linb_t = dt("linb", (P, cfg.RESC), BF16, kind="ExternalInput")
        resout_t = dt("resout", (NLOC, cfg.RESC), BF16, kind="ExternalOutput")
    if cfg.add_residual:
        resin_t = dt("resin", (NLOC, cfg.RESC), BF16, kind="ExternalInput")
    out_t = dt("out", (NLOC, CH), BF16, kind="ExternalOutput")
    xlc_ts = [dt(f"xlc{i}", (cfg.ROWPADS[i], HC), BF16) for i in range(NSUB)]

    with tile.TileContext(nc) as tc, ExitStack() as ctx, \
            nc.allow_low_precision(reason="bf16 softmax scores within 2e-2 tol"):
        cpool = ctx.enter_context(tc.tile_pool(name="const", bufs=1))
        xt_pool = ctx.enter_context(tc.tile_pool(name="xt", bufs=3))
        cp_pool = ctx.enter_context(tc.tile_pool(name="cp", bufs=2))
        g_pool = ctx.enter_context(tc.tile_pool(name="g", bufs=3))
        ohs_pool = ctx.enter_context(tc.tile_pool(name="ohs", bufs=3))
        lr_pool = ctx.enter_context(tc.tile_pool(name="lr", bufs=6))
        sm_pool = ctx.enter_context(tc.tile_pool(name="sm", bufs=6))
        ps_pool = ctx.enter_context(tc.tile_pool(name="ps", bufs=2, space="PSUM"))
        psu_pool = ctx.enter_context(tc.tile_pool(name="psu", bufs=2, space="PSUM"))
        psd_pool = ctx.enter_context(tc.tile_pool(name="psd", bufs=1, space="PSUM"))
        if cfg.KB:
            gt_pool = ctx.enter_context(tc.tile_pool(name="gt", bufs=3))
            pse_pool = ctx.enter_context(tc.tile_pool(name="pse", bufs=1, space="PSUM"))

        # ---- constants ----
        wl_sb = cpool.tile([P, HC], BF16)
        nc.sync.dma_start(out=wl_sb[:], in_=wl_t[:, :])
        wr_sb = cpool.tile([Fin, HC], BF16)
        nc.sync.dma_start(out=wr_sb[:], in_=wr_t[:, :])
        attb_sb = cpool.tile([P, 2 * HC], BF16)
        nc.sync.dma_start(out=attb_sb[:], in_=attb_t[:, :])
        if cfg.KB:
            attT_sb = cpool.tile([P, cfg.NBLK * H], BF16)
            nc.sync.dma_start(out=attT_sb[:], in_=attT_t[:, :])
        bbc_sb = cpool.tile([P, CH], BF16)
        nc.sync.dma_start(out=bbc_sb[:], in_=bbc_t[:, :])
        ident_sb = cpool.tile([P, P], BF16)
        make_identity(nc, ident_sb[:])
        idx16_sb = cpool.tile([P, NT * IDXW], I16)
        nc.sync.dma_start(out=idx16_sb[:], in_=idx16_t[:, :])
        xtloc_sb = cpool.tile([Fin, NLOC], BF16)
        nc.sync.dma_start(out=xtloc_sb[:], in_=xTloc_t[:, :])
        if cfg.calc_residual:
            linw_sb = cpool.tile([Fin, cfg.RESC], BF16)
            nc.sync.dma_start(out=linw_sb[:], in_=linw_t[:, :])
            linb_sb = cpool.tile([P, cfg.RESC], BF16)
            nc.sync.dma_start(out=linb_sb[:], in_=linb_t[:, :])
            res_acc = cpool.tile([P, NT * cfg.RESC], BF16)
        if cfg.add_residual:
            res_sb = cpool.tile([P, NT * cfg.RESC], BF16)
            nc.sync.dma_start(
                out=res_sb[:].rearrange("p (t c) -> p t c", t=NT),
                in_=resin_t[:, :].rearrange("(t p) c -> p t c", p=P),
            )
        h_acc = cpool.tile([P, NT * CH], BF16)

        # ---- phase 1 chunk emitter (interleaved with tile groups) ----
        flip = [0]

        def emit_chunks(g):
            xTc_t, xlc_t, rows = xTc_ts[g], xlc_ts[g], cfg.ROWPADS[g]
            c0 = 0
            while c0 < rows:
                csz = min(cfg.chunk, rows - c0)
                xt_sb = xt_pool.tile([Fin, csz], BF16, tag="xt")
                nc.sync.dma_start(out=xt_sb[:], in_=xTc_t[:, c0:c0 + csz])
                nj = csz // P
                ob = cp_pool.tile([P, nj * HC], BF16, tag="cp")
                for j in range(nj):
                    ps = psu_pool.tile([P, HC], F32, tag="U")
                    nc.tensor.matmul(ps[:], lhsT=xt_sb[:, j * P:(j + 1) * P],
                                     rhs=wl_sb[:Fin, :], start=True, stop=True)
                    dst = ob[:, j * HC:(j + 1) * HC]
                    if flip[0] % 3 == 0:
                        nc.vector.tensor_copy(dst, ps[:])
                    else:
                        nc.scalar.copy(dst, ps[:])
                    flip[0] += 1
                nc.sync.dma_start(
                    out=xlc_t[c0:c0 + csz, :].rearrange("(j p) c -> p j c", p=P),
                    in_=ob[:].rearrange("p (j c) -> p j c", j=nj))
                c0 += csz

        # ---- per-tile emitters ----
        import bisect
        KB, NBLK = cfg.KB, cfg.NBLK

        def pass1(t):
            grp = bisect.bisect_right(cfg.TB, t) - 1
            src_tab = xlc_ts[grp]
            ohs = ohs_pool.tile([P, OHW], BF16, tag="ohs")
            nc.sync.dma_start(out=ohs[:], in_=ohall_t[:, t * OHW:(t + 1) * OHW])
            xlg = g_pool.tile([P, NB * HC], BF16, tag="g")
            nc.gpsimd.dma_gather(
                out_ap=xlg[:].rearrange("p (b c) -> p b c", b=NB),
                in_ap=src_tab[:, :],
                idxs_ap=idx16_sb[:, t * IDXW:(t + 1) * IDXW],
                num_idxs=NIDX,
                num_idxs_reg=NIDX,
                elem_size=HC,
                single_packet=False,
            )
            if KB:
                xcm_tab = xcm_ts[grp]
                xgT = gt_pool.tile([P, KB * P], BF16, tag="gt")
                nc.gpsimd.dma_gather(
                    out_ap=xgT[:].rearrange("p (k e) -> p k e", k=1),
                    in_ap=xcm_tab[:, :],
                    idxs_ap=idx16_sb[:, t * IDXW:t * IDXW + KB * P // 16],
                    num_idxs=KB * P,
                    num_idxs_reg=KB * P,
                    elem_size=P,
                    transpose=True,
                    single_packet=False,
                )
            if cfg.calc_residual:
                ps2 = psd_pool.tile([P, cfg.RESC], F32, tag="D")
                nc.tensor.matmul(ps2[:], lhsT=xtloc_sb[:, t * P:(t + 1) * P],
                                 rhs=linw_sb[:], start=True, stop=True)
                nc.vector.tensor_tensor(
                    res_acc[:, t * cfg.RESC:(t + 1) * cfg.RESC],
                    ps2[:], linb_sb[:], op=ALU.add)
            psx = psu_pool.tile([P, HC], F32, tag="U")
            nc.tensor.matmul(psx[:], lhsT=xtloc_sb[:, t * P:(t + 1) * P],
                             rhs=wr_sb[:], start=True, stop=True)
            xrt = lr_pool.tile([P, HC], BF16, tag="xrt")
            nc.scalar.copy(xrt[:], psx[:])

            eacc = sm_pool.tile([P, NB * H], BF16, tag="eacc")
            exa = sm_pool.tile([P, NB * H], BF16, tag="exa")
            # oh layout per tile: [ohT(NB blocks) | oh(NB blocks)]

            def score_b_mms(bb, nsub):
                sT = ps_pool.tile([P, 2 * HC], F32, tag="mm2")
                for s_ in range(nsub):
                    b = bb + s_
                    for k in range(NBLK):
                        out = sT[:, s_ * HC + k * P:s_ * HC + (k + 1) * P]
                        nc.tensor.matmul(out, lhsT=xrt[:, k * P:(k + 1) * P],
                                         rhs=ohs[:, b * P:(b + 1) * P],
                                         start=True, stop=False)
                        nc.tensor.matmul(out, lhsT=wl_sb[:, k * P:(k + 1) * P],
                                         rhs=xgT[:, b * P:(b + 1) * P],
                                         start=False, stop=True)
                lrT = lr_pool.tile([P, 2 * HC], BF16, tag="lr")
                nc.scalar.activation(lrT[:, :nsub * HC], sT[:, :nsub * HC],
                                     AF.Prelu, alpha=cfg.neg_slope)
                return lrT

            def score_b_red(bb, nsub, lrT, eTall):
                for s_ in range(nsub):
                    b = bb + s_
                    for k in range(NBLK):
                        nc.tensor.matmul(
                            eTall[:H, b * P:(b + 1) * P],
                            lhsT=attT_sb[:, k * H:(k + 1) * H],
                            rhs=lrT[:, s_ * HC + k * P:s_ * HC + (k + 1) * P],
                            start=(k == 0), stop=(k == NBLK - 1))

            def score_d(bb, nsub):
                s2 = ps_pool.tile([P, 2 * HC], F32, tag="mm2")
                for s_ in range(nsub):
                    b = bb + s_
                    nc.tensor.matmul(s2[:, s_ * HC:(s_ + 1) * HC],
                                     lhsT=ohs[:, b * P:(b + 1) * P],
                                     rhs=xrt[:], start=True, stop=False)
                    nc.tensor.matmul(s2[:, s_ * HC:(s_ + 1) * HC],
                                     lhsT=ident_sb[:],
                                     rhs=xlg[:, b * HC:(b + 1) * HC],
                                     start=False, stop=True)
                w = nsub * HC
                lr = lr_pool.tile([P, 2 * HC], BF16, tag="lr")
                nc.scalar.activation(lr[:, :w], s2[:, :w], AF.Prelu,
                                     alpha=cfg.neg_slope)
                nc.vector.tensor_tensor(lr[:, :w], lr[:, :w], attb_sb[:, :w],
                                        op=ALU.mult)
                half = sm_pool.tile([P, HC], BF16, tag="half")
                lrv = lr[:].rearrange("p (s c) -> p s c", c=HC)
                nc.vector.tensor_tensor(
                    half[:, :w // 2].rearrange("p (s c) -> p s c", c=HC // 2),
                    lrv[:, :nsub, :HC // 2], lrv[:, :nsub, HC // 2:],
                    op=ALU.add)
                nc.vector.tensor_reduce(
                    eacc[:, bb * H:(bb + nsub) * H],
                    half[:, :w // 2].rearrange(
                        "p (s c h) -> p s h c", s=nsub, h=H),
                    axis=AX.X, op=ALU.add)

            if KB:
                eTat = pse_pool.tile([P, KB * P], F32, tag="eT", bufs=1)
                bpairs = []
                bb = 0
                while bb < KB:
                    nsub = min(2, KB - bb)
                    bpairs.append((bb, nsub, score_b_mms(bb, nsub)))
                    bb += nsub
                bb = KB
                if bb < NB:          # one D pair between sT-mms and eT-mms
                    nsub = min(2, NB - bb)
                    score_d(bb, nsub)
                    bb += nsub
                for pb, pn, lrT in bpairs:
                    score_b_red(pb, pn, lrT, eTat)
            else:
                bb = KB
            while bb < NB:
                nsub = min(2, NB - bb)
                score_d(bb, nsub)
                bb += nsub
            if KB:
                exT = sm_pool.tile([P, KB * P], BF16, tag="exT")
                nc.scalar.activation(exT[:H, :], eTat[:H, :], AF.Exp)
                exppt = psu_pool.tile([P, KB * H], BF16, tag="U")
                for b in range(KB):
                    nc.tensor.transpose(exppt[:, b * H:(b + 1) * H],
                                        exT[:H, b * P:(b + 1) * P],
                                        ident_sb[:H, :H])
                nc.vector.tensor_copy(exa[:, :KB * H], exppt[:])
            if KB < NB:
                nc.scalar.activation(exa[:, KB * H:], eacc[:, KB * H:], AF.Exp)
            return dict(t=t, ohs=ohs, xlg=xlg, exa=exa)

        def pass2(st):
            t, ohs, xlg, exa = st['t'], st['ohs'], st['xlg'], st['exa']
            U = psu_pool.tile([P, HC], F32, tag="U")
            D = psd_pool.tile([P, H], F32, tag="D")
            bb = 0
            while bb < NB:
                nsub = min(2, NB - bb)
                xb = xlg[:, bb * HC:(bb + nsub) * HC].rearrange(
                    "p (s c h) -> p s c h", s=nsub, h=H)
                exm = exa[:, bb * H:(bb + nsub) * H].rearrange(
                    "p (s o h) -> p s o h", s=nsub, o=1).broadcast_to(
                    [P, nsub, CH, H])
                nc.vector.tensor_tensor(xb, xb, exm, op=ALU.mult)
                for s_ in range(nsub):
                    b = bb + s_
                    nc.tensor.matmul(
                        U[:], lhsT=ohs[:, (NB + b) * P:(NB + b + 1) * P],
                        rhs=xlg[:, b * HC:(b + 1) * HC],
                        start=(b == 0), stop=(b == NB - 1))
                    nc.tensor.matmul(
                        D[:], lhsT=ohs[:, (NB + b) * P:(NB + b + 1) * P],
                        rhs=exa[:, b * H:(b + 1) * H],
                        start=(b == 0), stop=(b == NB - 1))
                bb += nsub
            # epilogue (pad dst slots: D=0 -> inf/NaN, discarded by host)
            rcp = sm_pool.tile([P, H], F32, tag="rcp")
            nc.vector.reciprocal(rcp[:], D[:])
            rcpb = sm_pool.tile([P, H], BF16, tag="rcpb")
            nc.vector.tensor_scalar_mul(rcpb[:], rcp[:], 1.0 / H)
            au = lr_pool.tile([P, HC], BF16, tag="au")
            auv = au[:].rearrange("p (c h) -> p c h", h=H)
            nc.vector.tensor_tensor(
                auv, U[:].rearrange("p (c h) -> p c h", h=H),
                rcpb[:].rearrange("p (o h) -> p o h", o=1).broadcast_to(
                    [P, CH, H]),
                op=ALU.mult)
            f1 = sm_pool.tile([P, CH * H // 2], BF16, tag="f1")
            nc.vector.tensor_tensor(
                f1[:].rearrange("p (c h) -> p c h", h=H // 2),
                auv[:, :, :H // 2], auv[:, :, H // 2:], op=ALU.add)
            f1v = f1[:].rearrange("p (c h) -> p c h", h=H // 2)
            f2 = sm_pool.tile([P, CH * H // 4], BF16, tag="f2")
            nc.vector.tensor_tensor(
                f2[:].rearrange("p (c h) -> p c h", h=H // 4),
                f1v[:, :, :H // 4], f1v[:, :, H // 4:], op=ALU.add)
            f2v = f2[:].rearrange("p (c h) -> p c h", h=H // 4)
            t1 = sm_pool.tile([P, CH], BF16, tag="t1")
            nc.vector.tensor_tensor(
                t1[:].rearrange("p (c h) -> p c h", h=1),
                f2v[:, :, :H // 8], f2v[:, :, H // 8:], op=ALU.add)
            hslice = h_acc[:, t * CH:(t + 1) * CH]
            if cfg.add_residual:
                nc.gpsimd.tensor_tensor(
                    t1[:], t1[:], res_sb[:, t * cfg.RESC:(t + 1) * cfg.RESC],
                    op=ALU.add)
            if cfg.relu:
                nc.gpsimd.tensor_tensor(t1[:], t1[:], bbc_sb[:], op=ALU.add)
                nc.scalar.activation(hslice, t1[:], AF.Relu)
            else:
                nc.gpsimd.tensor_tensor(hslice, t1[:], bbc_sb[:], op=ALU.add)

        # ---- interleaved emission: chunks lead their tile groups by 2 ----
        emit_chunks(0)
        if NSUB > 1:
            emit_chunks(1)
        prev = None
        for t in range(NT):
            g = bisect.bisect_right(cfg.TB, t) - 1
            if t == cfg.TB[g] and g + 2 < NSUB:
                emit_chunks(g + 2)
            st = pass1(t)
            if prev is not None:
                pass2(prev)
            prev = st
        pass2(prev)

        # ---- final stores ----
        nc.sync.dma_start(
            out=out_t[:, :].rearrange("(t p) c -> p t c", p=P),
            in_=h_acc[:].rearrange("p (t c) -> p t c", t=NT),
        )
        if cfg.calc_residual:
            nc.sync.dma_start(
                out=resout_t[:, :].rearrange("(t p) c -> p t c", p=P),
                in_=res_acc[:].rearrange("p (t c) -> p t c", t=NT),
            )
    return nc


# ---------------------------------------------------------------------------
# Host-side preprocessing
# ---------------------------------------------------------------------------

def pack_nodes(edge_index: np.ndarray, n: int, ncores: int, nt: int):
    """Assign each node to (core, tile, slot) balancing per-tile edge load.

    Returns (node_core, node_tile, node_slot, per-tile edge caps honored NB).
    """
    import heapq
    deg = np.bincount(edge_index[1], minlength=n).astype(np.int64) + 1
    nbins = ncores * nt
    for NB in (9, 10, 11):
        cap = NB * P
        order = np.argsort(-deg, kind="stable")
        loads = np.zeros(nbins, np.int64)
        slots = np.zeros(nbins, np.int32)
        node_bin = np.full(n, -1, np.int32)
        heap = [(0, b) for b in range(nbins)]
        heapq.heapify(heap)
        ok = True
        skipped = []
        for node in order:
            d = int(deg[node])
            tried = []
            placed = False
            while heap:
                load, b = heapq.heappop(heap)
                if load != loads[b] or slots[b] >= P:
                    continue   # stale or full
                if load + d <= cap:
                    node_bin[node] = b
                    loads[b] += d
                    slots[b] += 1
                    if slots[b] < P:
                        heapq.heappush(heap, (loads[b], b))
                    placed = True
                    break
                tried.append((load, b))
            for item in tried:
                heapq.heappush(heap, item)
            if not placed:
                ok = False
                break
        if ok:
            bins = node_bin
            node_core = bins // nt
            node_tile = bins % nt
            node_slot = np.zeros(n, np.int32)
            for b in range(nbins):
                idx = np.where(bins == b)[0]
                node_slot[idx] = np.arange(len(idx), dtype=np.int32)
            return node_core.astype(np.int32), node_tile.astype(np.int32), \
                node_slot, NB
    raise RuntimeError("packing failed")


def preprocess_edges(edge_index: np.ndarray, n: int, ncores: int,
                     nsub: int = 4, int16_cap: int = 32000):
    """Shard edges by packed dst; build per-core idx16 / oh streams.

    Returns (metas, layout). metas[c]:
      idx16  [128, NT*NB*8]  int16
      ohall  [128, NT*2*NB*128]  float32 (cast to bf16 later)
      usrc   [nsub] unique source-node arrays
      nodes  [NT*128] int64 node id per (tile,slot), -1 for empty
    """
    nloc = -(-n // ncores)
    NT = -(-nloc // P)
    node_core, node_tile, node_slot, NB = pack_nodes(edge_index, n, ncores, NT)
    TB = tuple(round(i * NT / nsub) for i in range(nsub + 1))
    NIDX = NB * P
    IDXW = NIDX // 16

    loops = np.arange(n, dtype=np.int64)
    src = np.concatenate([edge_index[0].astype(np.int64), loops])
    dst = np.concatenate([edge_index[1].astype(np.int64), loops])
    ecore = node_core[dst]
    etile = node_tile[dst]
    eslot = node_slot[dst]

    metas = []
    rowmax = [0] * nsub
    for c in range(ncores):
        sel = ecore == c
        s_c, t_c, d_c = src[sel], etile[sel], eslot[sel]
        order = np.argsort(t_c, kind="stable")
        s_c, t_c, d_c = s_c[order], t_c[order], d_c[order]
        tcnt = np.bincount(t_c, minlength=NT)
        tstart = np.concatenate([[0], np.cumsum(tcnt)])

        idxflat = np.zeros((NT, NIDX), np.int64)
        oh = np.zeros((NT, NB, P, P), np.float32)      # [t, b, e, d]
        usrcs = []
        for sub in range(nsub):
            tlo, thi = TB[sub], TB[sub + 1]
            allsrc = s_c[tstart[tlo]:tstart[thi]]
            usrc, inv = np.unique(allsrc, return_inverse=True)
            assert len(usrc) < int16_cap, f"int16 cap exceeded: {len(usrc)}"
            usrcs.append(usrc)
            comp = np.zeros(len(s_c), np.int64)
            comp[tstart[tlo]:tstart[thi]] = inv
            for t in range(tlo, thi):
                ne = tcnt[t]
                e0 = tstart[t]
                idxflat[t, :ne] = comp[e0:e0 + ne]
                eix = np.arange(ne)
                oh[t, eix // P, eix % P, d_c[e0:e0 + ne]] = 1.0
        ohT = np.transpose(oh, (0, 1, 3, 2))           # [t, b, d, e]
        # per tile: [ohT blocks | oh blocks] -> [t, 2, NB, P(row), P(col)]
        ohcat = np.stack([ohT, oh], axis=1)            # [t, 2, b, row, col]
        ohall = np.ascontiguousarray(
            np.transpose(ohcat, (3, 0, 1, 2, 4)).reshape(P, NT * 2 * NB * P))

        w = idxflat.reshape(NT, IDXW, 16).transpose(0, 2, 1)
        idx16 = np.tile(w, (1, 8, 1)).transpose(1, 0, 2).reshape(P, NT * IDXW)

        nodes = np.full(NT * P, -1, np.int64)
        nsel = np.where(node_core == c)[0]
        nodes[node_tile[nsel] * P + node_slot[nsel]] = nsel
        metas.append(dict(idx16=idx16.astype(np.int16), ohall=ohall,
                          usrc=usrcs, nodes=nodes))
        for sub in range(nsub):
            rowmax[sub] = max(rowmax[sub], len(metas[c]['usrc'][sub]))
    rowpad = tuple(max(P, -(-r // P) * P) for r in rowmax)
    layout = dict(NT=NT, NB=NB, TB=TB, nloc_pad=NT * P, ROWPADS=rowpad)
    return metas, layout


# ---------------------------------------------------------------------------
# Top-level kernel entry: full inputs -> full output, 8 NeuronCores
# ---------------------------------------------------------------------------
import ml_dtypes

_BF16NP = ml_dtypes.bfloat16
N_NODES = 50000
F_IN = 128
N_HEADS = 8
C_HID = 64
K_OUT = 32
NCORES = 8

_compiled_cache = {}


def layer_cfgs(lay):
    NB = lay['NB']
    common = dict(NT=lay['NT'], NB=NB, TB=tuple(lay['TB']),
                  ROWPADS=tuple(lay['ROWPADS']))
    cfg1 = LayerCfg(Fin=F_IN, H=N_HEADS, CH=C_HID, relu=True,
                    calc_residual=True, add_residual=False,
                    KB=min(4, NB), **common)
    cfg2 = LayerCfg(Fin=C_HID, H=N_HEADS, CH=K_OUT, relu=False,
                    calc_residual=False, add_residual=True,
                    KB=min(6, NB), **common)
    return cfg1, cfg2


def _attT_const(att_cm, H):
    """[P, NBLK*H]: attT[p, k*H+h] = att_cm[k*128+p] iff (k*128+p)%H==h."""
    HC = len(att_cm)
    NBLK = HC // P
    out = np.zeros((P, NBLK * H), np.float32)
    j = np.arange(HC)
    out[j % P, (j // P) * H + (j % H)] = att_cm
    return out.astype(_BF16NP)


def _build_programs(lay):
    key = (lay['NT'], lay['NB'], tuple(lay['TB']), tuple(lay['ROWPADS']))
    if key in _compiled_cache:
        return _compiled_cache[key]
    cfg1, cfg2 = layer_cfgs(lay)
    ncs = []
    for cfg in (cfg1, cfg2):
        nc = bacc.Bacc("TRN2", target_bir_lowering=False, debug=False,
                       num_devices=NCORES)
        build_layer(nc, cfg)
        nc.compile()
        ncs.append(nc)
    _compiled_cache[key] = tuple(ncs)
    return _compiled_cache[key]


def _ch_major_cols(H, C):
    """col j holds original column h*C+c with j = c*H + h."""
    j = np.arange(H * C)
    c, h = j // H, j % H
    return h * C + c


def _compact_tables(xfull, m, Fin, lay):
    outs = []
    for s, rp in enumerate(lay['ROWPADS']):
        xc = np.zeros((rp, Fin), np.float32)
        u = m['usrc'][s]
        xc[:len(u)] = xfull[u]
        outs.append(np.ascontiguousarray(xc.T).astype(_BF16NP))
    return outs


def _compact_rm(xfull, m, lay):
    """Row-major compact tables, feature dim zero-padded to 128."""
    outs = []
    for s, rp in enumerate(lay['ROWPADS']):
        xc = np.zeros((rp, P), np.float32)
        u = m['usrc'][s]
        xc[:len(u), :xfull.shape[1]] = xfull[u]
        outs.append(xc.astype(_BF16NP))
    return outs


def _pad_rows(w, rows=P):
    out = np.zeros((rows, w.shape[1]), np.float32)
    out[:w.shape[0]] = w
    return out


def _local_table(xfull, Fin, m):
    nodes = m['nodes']
    xl = np.zeros((len(nodes), Fin), np.float32)
    valid = nodes >= 0
    xl[valid] = xfull[nodes[valid]]
    return np.ascontiguousarray(xl.T).astype(_BF16NP)


def kernel(x, edge_index, xyz, lin1_w, lin1_b, wl1, wr1, att1, b1,
           wl2, wr2, att2, b2):
    from concourse.bass_utils import run_bass_kernel_spmd

    x = np.asarray(x, dtype=np.float32)
    edge_index = np.asarray(edge_index)
    metas, lay = preprocess_edges(edge_index, N_NODES, NCORES)
    nc1, nc2 = _build_programs(lay)

    perm1 = _ch_major_cols(N_HEADS, C_HID)
    perm2 = _ch_major_cols(N_HEADS, K_OUT)
    wl1_cm = np.asarray(wl1, np.float32)[:, perm1]
    wr1_cm = np.asarray(wr1, np.float32)[:, perm1]
    att1_cm = np.asarray(att1, np.float32).reshape(-1)[perm1]
    wl2_cm = np.asarray(wl2, np.float32)[:, perm2]
    wr2_cm = np.asarray(wr2, np.float32)[:, perm2]
    att2_cm = np.asarray(att2, np.float32).reshape(-1)[perm2]

    attb1 = np.tile(att1_cm.reshape(1, -1), (P, 2)).astype(_BF16NP)
    attb2 = np.tile(att2_cm.reshape(1, -1), (P, 2)).astype(_BF16NP)
    b1bc = np.tile(np.asarray(b1, np.float32).reshape(1, -1), (P, 1)).astype(_BF16NP)
    b2bc = np.tile(np.asarray(b2, np.float32).reshape(1, -1), (P, 1)).astype(_BF16NP)
    linbbc = np.tile(np.asarray(lin1_b, np.float32).reshape(1, -1),
                     (P, 1)).astype(_BF16NP)

    cfg1, cfg2 = layer_cfgs(lay)
    attT1 = _attT_const(att1_cm, N_HEADS)
    attT2 = _attT_const(att2_cm, N_HEADS)

    in_maps1 = []
    for c in range(NCORES):
        m = metas[c]
        xtc = _compact_tables(x, m, F_IN, lay)
        im = dict(
            xTloc=_local_table(x, F_IN, m),
            wl=_pad_rows(wl1_cm).astype(_BF16NP), wr=wr1_cm.astype(_BF16NP),
            attb=attb1, bbc=b1bc, idx16=m['idx16'],
            ohall=m['ohall'].astype(_BF16NP),
            linw=np.asarray(lin1_w, np.float32).astype(_BF16NP), linb=linbbc)
        for i, tab in enumerate(xtc):
            im[f'xTc{i}'] = tab
        if cfg1.KB:
            im['attT'] = attT1
            for i, tab in enumerate(_compact_rm(x, m, lay)):
                im[f'xcm{i}'] = tab
        in_maps1.append(im)
    res1 = run_bass_kernel_spmd(nc1, in_maps1, core_ids=list(range(NCORES)))

    h_full = np.zeros((N_NODES, C_HID), np.float32)
    res_full = np.zeros((N_NODES, K_OUT), np.float32)
    for c in range(NCORES):
        nodes = metas[c]['nodes']
        valid = nodes >= 0
        h_full[nodes[valid]] = np.asarray(
            res1.results[c]["out"], np.float32)[valid]
        res_full[nodes[valid]] = np.asarray(
            res1.results[c]["resout"], np.float32)[valid]

    in_maps2 = []
    for c in range(NCORES):
        m = metas[c]
        htc = _compact_tables(h_full, m, C_HID, lay)
        resin = np.zeros((lay['nloc_pad'], K_OUT), np.float32)
        nodes = m['nodes']
        valid = nodes >= 0
        resin[valid] = res_full[nodes[valid]]
        im = dict(
            xTloc=_local_table(h_full, C_HID, m),
            wl=_pad_rows(wl2_cm).astype(_BF16NP), wr=wr2_cm.astype(_BF16NP),
            attb=attb2, bbc=b2bc, idx16=m['idx16'],
            ohall=m['ohall'].astype(_BF16NP),
            resin=resin.astype(_BF16NP))
        for i, tab in enumerate(htc):
            im[f'xTc{i}'] = tab
        if cfg2.KB:
            im['attT'] = attT2
            for i, tab in enumerate(_compact_rm(h_full, m, lay)):
                im[f'xcm{i}'] = tab
        in_maps2.append(im)
    res2 = run_bass_kernel_spmd(nc2, in_maps2, core_ids=list(range(NCORES)))

    out = np.zeros((N_NODES, K_OUT), np.float32)
    for c in range(NCORES):
        nodes = metas[c]['nodes']
        valid = nodes >= 0
        out[nodes[valid]] = np.asarray(
            res2.results[c]["out"], np.float32)[valid]
    return out



# revision 2
# speedup vs baseline: 1.0782x; 1.0782x over previous
"""Bass/Tile GATv2 kernel for TRN2, dst-sharded across 8 cores.

Two GATv2 layers (+linear residual), each run as one SPMD program over 8
NeuronCores. Host side: nodes are bin-packed into (core, tile, slot) so every
128-dst tile has nearly equal edge count (NB blocks of 128 edges); one-hot
edge->dst matrices (oh) and their transposes (ohT) are precomputed on host and
streamed to the device; source features are compacted per subphase so int16
gather indices suffice.

Device side per dst tile (all feature tensors channel-major: col = c*H + h):
  - DMA oh/ohT stream + dma_gather of xl rows (from a DRAM table built in
    phase 1 as x_compact @ wl)
  - xr = xtloc @ wr (PE) per tile
  - pass 1 per 128-edge block: s = ohT^T@xr + I@xlg (PE, PSUM), lr =
    prelu(s) (scalar), lr *= att (DVE 2x), fold+reduce -> e[:,8] (DVE)
  - one exp over all blocks' e (scalar)
  - pass 2 per block: xlw = xlg * exp-bcast (DVE 2x, ch-major), U += oh^T@xlw,
    D += oh^T@ex (PE)
  - epilogue: out = mean_h(U/D) + bias [+res / relu]
"""
from contextlib import ExitStack
from dataclasses import dataclass

import numpy as np

import concourse.bass as bass
import concourse.tile as tile
from concourse import bacc, mybir
from concourse.masks import make_identity

F32 = mybir.dt.float32
BF16 = mybir.dt.bfloat16
I16 = mybir.dt.int16
AF = mybir.ActivationFunctionType
ALU = mybir.AluOpType
AX = mybir.AxisListType
P = 128


@dataclass
class LayerCfg:
    Fin: int            # contraction dim of input features (128 L1, 64 L2)
    H: int              # heads
    CH: int             # per-head channels (64 L1, 32 L2)
    NT: int             # dst tiles per core
    NB: int             # edge blocks per tile (uniform across cores/tiles)
    TB: tuple           # tile-group boundaries, len NSUB+1
    ROWPADS: tuple      # compact-table rows per subphase (128-mult, uniform)
    relu: bool = False
    calc_residual: bool = False
    add_residual: bool = False
    RESC: int = 32
    neg_slope: float = 0.2
    chunk: int = 2048   # xTc column chunk for phase 1
    KB: int = 0         # blocks per tile scored via PE (transposed path)
    att_pool: int = 0   # score_d pairs (from the end) with att-mult+fold on Pool
    b_pool: int = 0     # pass2 B-mult pairs (from the end) on Pool (SBUF-only)
    cpsplit: tuple = (0, 1, 1)   # phase-1 copy engine rotation (0=DVE,1=Act)

    @property
    def HC(self):
        return self.H * self.CH

    @property
    def NBLK(self):
        return self.HC // P


def build_layer(nc: bacc.Bacc, cfg: LayerCfg):
    HC, H, CH, Fin, NT, NB = cfg.HC, cfg.H, cfg.CH, cfg.Fin, cfg.NT, cfg.NB
    NLOC = NT * P
    NIDX = NB * P
    IDXW = NIDX // 16
    OHW = 2 * NB * P          # oh+ohT columns per tile

    dt = nc.dram_tensor
    NSUB = len(cfg.ROWPADS)
    xTc_ts = [dt(f"xTc{i}", (Fin, cfg.ROWPADS[i]), BF16, kind="ExternalInput")
              for i in range(NSUB)]
    xTloc_t = dt("xTloc", (Fin, NLOC), BF16, kind="ExternalInput")
    if cfg.KB:
        # row-major compact inputs (feature dim zero-padded to 128) for the
        # transposed B-path gathers
        xcm_ts = [dt(f"xcm{i}", (cfg.ROWPADS[i], P), BF16,
                     kind="ExternalInput") for i in range(NSUB)]
    wl_t = dt("wl", (P, HC), BF16, kind="ExternalInput")
    wr_t = dt("wr", (Fin, HC), BF16, kind="ExternalInput")
    attb_t = dt("attb", (P, 2 * HC), BF16, kind="ExternalInput")
    if cfg.KB:
        attT_t = dt("attT", (P, cfg.NBLK * H), BF16, kind="ExternalInput")
    bbc_t = dt("bbc", (P, CH), BF16, kind="ExternalInput")
    idx16_t = dt("idx16", (P, NT * IDXW), I16, kind="ExternalInput")
    ohall_t = dt("ohall", (P, NT * OHW), BF16, kind="ExternalInput")
    if cfg.calc_residual:
        linw_t = dt("linw", (Fin, cfg.RESC), BF16, kind="ExternalInput")
        linb_t = dt("linb", (P, cfg.RESC), BF16, kind="ExternalInput")
        resout_t = dt("resout", (NLOC, cfg.RESC), BF16, kind="ExternalOutput")
    if cfg.add_residual:
        resin_t = dt("resin", (NLOC, cfg.RESC), BF16, kind="ExternalInput")
    out_t = dt("out", (NLOC, CH), BF16, kind="ExternalOutput")
    xlc_ts = [dt(f"xlc{i}", (cfg.ROWPADS[i], HC), BF16) for i in range(NSUB)]

    with tile.TileContext(nc) as tc, ExitStack() as ctx, \
            nc.allow_low_precision(reason="bf16 softmax scores within 2e-2 tol"):
        cpool = ctx.enter_context(tc.tile_pool(name="const", bufs=1))
        xt_pool = ctx.enter_context(tc.tile_pool(name="xt", bufs=3))
        cp_pool = ctx.enter_context(tc.tile_pool(name="cp", bufs=2))
        g_pool = ctx.enter_context(tc.tile_pool(name="g", bufs=3))
        ohs_pool = ctx.enter_context(tc.tile_pool(name="ohs", bufs=4))
        lr_pool = ctx.enter_context(tc.tile_pool(name="lr", bufs=6))
        sm_pool = ctx.enter_context(tc.tile_pool(name="sm", bufs=6))
        ps_pool = ctx.enter_context(tc.tile_pool(name="ps", bufs=2, space="PSUM"))
        psu_pool = ctx.enter_context(tc.tile_pool(name="psu", bufs=2, space="PSUM"))
        psd_pool = ctx.enter_context(tc.tile_pool(name="psd", bufs=1, space="PSUM"))
        if cfg.KB:
            gt_pool = ctx.enter_context(tc.tile_pool(name="gt", bufs=3))
            pse_pool = ctx.enter_context(tc.tile_pool(name="pse", bufs=1, space="PSUM"))

        # ---- constants ----
        wl_sb = cpool.tile([P, HC], BF16)
        nc.sync.dma_start(out=wl_sb[:], in_=wl_t[:, :])
        wr_sb = cpool.tile([Fin, HC], BF16)
        nc.sync.dma_start(out=wr_sb[:], in_=wr_t[:, :])
        attb_sb = cpool.tile([P, 2 * HC], BF16)
        nc.sync.dma_start(out=attb_sb[:], in_=attb_t[:, :])
        if cfg.KB:
            attT_sb = cpool.tile([P, cfg.NBLK * H], BF16)
            nc.sync.dma_start(out=attT_sb[:], in_=attT_t[:, :])
        bbc_sb = cpool.tile([P, CH], BF16)
        nc.sync.dma_start(out=bbc_sb[:], in_=bbc_t[:, :])
        ident_sb = cpool.tile([P, P], BF16)
        make_identity(nc, ident_sb[:])
        idx16_sb = cpool.tile([P, NT * IDXW], I16)
        nc.sync.dma_start(out=idx16_sb[:], in_=idx16_t[:, :])
        xtloc_sb = cpool.tile([Fin, NLOC], BF16)
        nc.sync.dma_start(out=xtloc_sb[:], in_=xTloc_t[:, :])
        if cfg.calc_residual:
            linw_sb = cpool.tile([Fin, cfg.RESC], BF16)
            nc.sync.dma_start(out=linw_sb[:], in_=linw_t[:, :])
            linb_sb = cpool.tile([P, cfg.RESC], BF16)
            nc.sync.dma_start(out=linb_sb[:], in_=linb_t[:, :])
            res_acc = cpool.tile([P, NT * cfg.RESC], BF16)
        if cfg.add_residual:
            res_sb = cpool.tile([P, NT * cfg.RESC], BF16)
            nc.sync.dma_start(
                out=res_sb[:].rearrange("p (t c) -> p t c", t=NT),
                in_=resin_t[:, :].rearrange("(t p) c -> p t c", p=P),
            )
        h_acc = cpool.tile([P, NT * CH], BF16)

        # ---- phase 1 chunk emitter (interleaved with tile groups) ----
        flip = [0]

        def emit_chunks(g):
            xTc_t, xlc_t, rows = xTc_ts[g], xlc_ts[g], cfg.ROWPADS[g]
            c0 = 0
            while c0 < rows:
                csz = min(cfg.chunk, rows - c0)
                xt_sb = xt_pool.tile([Fin, csz], BF16, tag="xt")
                nc.sync.dma_start(out=xt_sb[:], in_=xTc_t[:, c0:c0 + csz])
                nj = csz // P
                ob = cp_pool.tile([P, nj * HC], BF16, tag="cp")
                for j in range(nj):
                    ps = psu_pool.tile([P, HC], F32, tag="U")
                    nc.tensor.matmul(ps[:], lhsT=xt_sb[:, j * P:(j + 1) * P],
                                     rhs=wl_sb[:Fin, :], start=True, stop=True)
                    dst = ob[:, j * HC:(j + 1) * HC]
                    eng = cfg.cpsplit[flip[0] % len(cfg.cpsplit)]
                    if eng == 0:
                        nc.vector.tensor_copy(dst, ps[:])
                    elif eng == 1:
                        nc.scalar.copy(dst, ps[:])
                    else:
                        nc.gpsimd.tensor_copy(dst, ps[:])
                    flip[0] += 1
                nc.sync.dma_start(
                    out=xlc_t[c0:c0 + csz, :].rearrange("(j p) c -> p j c", p=P),
                    in_=ob[:].rearrange("p (j c) -> p j c", j=nj))
                c0 += csz

        # ---- per-tile emitters ----
        import bisect
        KB, NBLK = cfg.KB, cfg.NBLK

        def pass1(t):
            grp = bisect.bisect_right(cfg.TB, t) - 1
            src_tab = xlc_ts[grp]
            ohs = ohs_pool.tile([P, OHW], BF16, tag="ohs")
            nc.sync.dma_start(out=ohs[:], in_=ohall_t[:, t * OHW:(t + 1) * OHW])
            xlg = g_pool.tile([P, NB * HC], BF16, tag="g")
            nc.gpsimd.dma_gather(
                out_ap=xlg[:].rearrange("p (b c) -> p b c", b=NB),
                in_ap=src_tab[:, :],
                idxs_ap=idx16_sb[:, t * IDXW:(t + 1) * IDXW],
                num_idxs=NIDX,
                num_idxs_reg=NIDX,
                elem_size=HC,
                single_packet=False,
            )
            if KB:
                xcm_tab = xcm_ts[grp]
                xgT = gt_pool.tile([P, KB * P], BF16, tag="gt")
                nc.gpsimd.dma_gather(
                    out_ap=xgT[:].rearrange("p (k e) -> p k e", k=1),
                    in_ap=xcm_tab[:, :],
                    idxs_ap=idx16_sb[:, t * IDXW:t * IDXW + KB * P // 16],
                    num_idxs=KB * P,
                    num_idxs_reg=KB * P,
                    elem_size=P,
                    transpose=True,
                    single_packet=False,
                )
            if cfg.calc_residual:
                ps2 = psd_pool.tile([P, cfg.RESC], F32, tag="D")
                nc.tensor.matmul(ps2[:], lhsT=xtloc_sb[:, t * P:(t + 1) * P],
                                 rhs=linw_sb[:], start=True, stop=True)
                nc.vector.tensor_tensor(
                    res_acc[:, t * cfg.RESC:(t + 1) * cfg.RESC],
                    ps2[:], linb_sb[:], op=ALU.add)
            psx = psu_pool.tile([P, HC], F32, tag="U")
            nc.tensor.matmul(psx[:], lhsT=xtloc_sb[:, t * P:(t + 1) * P],
                             rhs=wr_sb[:], start=True, stop=True)
            xrt = lr_pool.tile([P, HC], BF16, tag="xrt")
            nc.scalar.copy(xrt[:], psx[:])

            eacc = sm_pool.tile([P, NB * H], BF16, tag="eacc")
            exa = sm_pool.tile([P, NB * H], BF16, tag="exa")
            # oh layout per tile: [ohT(NB blocks) | oh(NB blocks)]

            def npairs_d(bb):
                n = 0
                b2 = bb
                while b2 < NB:
                    n += 1
                    b2 += min(2, NB - b2)
                return n

            def score_b_mms(bb, nsub):
                sT = ps_pool.tile([P, 2 * HC], F32, tag="mm2")
                for s_ in range(nsub):
                    b = bb + s_
                    for k in range(NBLK):
                        out = sT[:, s_ * HC + k * P:s_ * HC + (k + 1) * P]
                        nc.tensor.matmul(out, lhsT=xrt[:, k * P:(k + 1) * P],
                                         rhs=ohs[:, b * P:(b + 1) * P],
                                         start=True, stop=False)
                        nc.tensor.matmul(out, lhsT=wl_sb[:, k * P:(k + 1) * P],
                                         rhs=xgT[:, b * P:(b + 1) * P],
                                         start=False, stop=True)
                lrT = lr_pool.tile([P, 2 * HC], BF16, tag="lr")
                nc.scalar.activation(lrT[:, :nsub * HC], sT[:, :nsub * HC],
                                     AF.Prelu, alpha=cfg.neg_slope)
                return lrT

            def score_b_red(bb, nsub, lrT, eTall):
                for s_ in range(nsub):
                    b = bb + s_
                    for k in range(NBLK):
                        nc.tensor.matmul(
                            eTall[:H, b * P:(b + 1) * P],
                            lhsT=attT_sb[:, k * H:(k + 1) * H],
                            rhs=lrT[:, s_ * HC + k * P:s_ * HC + (k + 1) * P],
                            start=(k == 0), stop=(k == NBLK - 1))

            def score_d(bb, nsub, pool_att=False):
                s2 = ps_pool.tile([P, 2 * HC], F32, tag="mm2")
                for s_ in range(nsub):
                    b = bb + s_
                    nc.tensor.matmul(s2[:, s_ * HC:(s_ + 1) * HC],
                                     lhsT=ohs[:, b * P:(b + 1) * P],
                                     rhs=xrt[:], start=True, stop=False)
                    nc.tensor.matmul(s2[:, s_ * HC:(s_ + 1) * HC],
                                     lhsT=ident_sb[:],
                                     rhs=xlg[:, b * HC:(b + 1) * HC],
                                     start=False, stop=True)
                w = nsub * HC
                lr = lr_pool.tile([P, 2 * HC], BF16, tag="lr")
                nc.scalar.activation(lr[:, :w], s2[:, :w], AF.Prelu,
                                     alpha=cfg.neg_slope)
                aeng = nc.gpsimd if pool_att else nc.vector
                aeng.tensor_tensor(lr[:, :w], lr[:, :w], attb_sb[:, :w],
                                   op=ALU.mult)
                half = sm_pool.tile([P, HC], BF16, tag="half")
                lrv = lr[:].rearrange("p (s c) -> p s c", c=HC)
                aeng.tensor_tensor(
                    half[:, :w // 2].rearrange("p (s c) -> p s c", c=HC // 2),
                    lrv[:, :nsub, :HC // 2], lrv[:, :nsub, HC // 2:],
                    op=ALU.add)
                nc.vector.tensor_reduce(
                    eacc[:, bb * H:(bb + nsub) * H],
                    half[:, :w // 2].rearrange(
                        "p (s c h) -> p s h c", s=nsub, h=H),
                    axis=AX.X, op=ALU.add)

            if KB:
                eTat = pse_pool.tile([P, KB * P], F32, tag="eT", bufs=1)
                bpairs = []
                bb = 0
                while bb < KB:
                    nsub = min(2, KB - bb)
                    bpairs.append((bb, nsub, score_b_mms(bb, nsub)))
                    bb += nsub
                bb = KB
                if bb < NB:          # one D pair between sT-mms and eT-mms
                    nsub = min(2, NB - bb)
                    score_d(bb, nsub, pool_att=(npairs_d(KB) <= cfg.att_pool))
                    bb += nsub
                for pb, pn, lrT in bpairs:
                    score_b_red(pb, pn, lrT, eTat)
            else:
                bb = KB
            while bb < NB:
                nsub = min(2, NB - bb)
                score_d(bb, nsub, pool_att=(npairs_d(bb) <= cfg.att_pool))
                bb += nsub
            if KB:
                exT = sm_pool.tile([P, KB * P], BF16, tag="exT")
                nc.scalar.activation(exT[:H, :], eTat[:H, :], AF.Exp)
                exppt = psu_pool.tile([P, KB * H], BF16, tag="U")
                for b in range(KB):
                    nc.tensor.transpose(exppt[:, b * H:(b + 1) * H],
                                        exT[:H, b * P:(b + 1) * P],
                                        ident_sb[:H, :H])
                nc.vector.tensor_copy(exa[:, :KB * H], exppt[:])
            if KB < NB:
                nc.scalar.activation(exa[:, KB * H:], eacc[:, KB * H:], AF.Exp)
            return dict(t=t, ohs=ohs, xlg=xlg, exa=exa)

        def pass2(st):
            t, ohs, xlg, exa = st['t'], st['ohs'], st['xlg'], st['exa']
            U = psu_pool.tile([P, HC], F32, tag="U")
            D = psd_pool.tile([P, H], F32, tag="D")
            bb = 0
            pleft = (NB + 1) // 2
            while bb < NB:
                nsub = min(2, NB - bb)
                xb = xlg[:, bb * HC:(bb + nsub) * HC].rearrange(
                    "p (s c h) -> p s c h", s=nsub, h=H)
                exm = exa[:, bb * H:(bb + nsub) * H].rearrange(
                    "p (s o h) -> p s o h", s=nsub, o=1).broadcast_to(
                    [P, nsub, CH, H])
                beng = nc.gpsimd if pleft <= cfg.b_pool else nc.vector
                beng.tensor_tensor(xb, xb, exm, op=ALU.mult)
                pleft -= 1
                for s_ in range(nsub):
                    b = bb + s_
                    nc.tensor.matmul(
                        U[:], lhsT=ohs[:, (NB + b) * P:(NB + b + 1) * P],
                        rhs=xlg[:, b * HC:(b + 1) * HC],
                        start=(b == 0), stop=(b == NB - 1))
                    nc.tensor.matmul(
                        D[:], lhsT=ohs[:, (NB + b) * P:(NB + b + 1) * P],
                        rhs=exa[:, b * H:(b + 1) * H],
                        start=(b == 0), stop=(b == NB - 1))
                bb += nsub
            # epilogue (pad dst slots: D=0 -> inf/NaN, discarded by host)
            rcp = sm_pool.tile([P, H], F32, tag="rcp")
            nc.vector.reciprocal(rcp[:], D[:])
            rcpb = sm_pool.tile([P, H], BF16, tag="rcpb")
            nc.vector.tensor_scalar_mul(rcpb[:], rcp[:], 1.0 / H)
            au = lr_pool.tile([P, HC], BF16, tag="au")
            auv = au[:].rearrange("p (c h) -> p c h", h=H)
            nc.vector.tensor_tensor(
                auv, U[:].rearrange("p (c h) -> p c h", h=H),
                rcpb[:].rearrange("p (o h) -> p o h", o=1).broadcast_to(
                    [P, CH, H]),
                op=ALU.mult)
            f1 = sm_pool.tile([P, CH * H // 2], BF16, tag="f1")
            nc.vector.tensor_tensor(
                f1[:].rearrange("p (c h) -> p c h", h=H // 2),
                auv[:, :, :H // 2], auv[:, :, H // 2:], op=ALU.add)
            f1v = f1[:].rearrange("p (c h) -> p c h", h=H // 2)
            f2 = sm_pool.tile([P, CH * H // 4], BF16, tag="f2")
            nc.vector.tensor_tensor(
                f2[:].rearrange("p (c h) -> p c h", h=H // 4),
                f1v[:, :, :H // 4], f1v[:, :, H // 4:], op=ALU.add)
            f2v = f2[:].rearrange("p (c h) -> p c h", h=H // 4)
            t1 = sm_pool.tile([P, CH], BF16, tag="t1")
            nc.vector.tensor_tensor(
                t1[:].rearrange("p (c h) -> p c h", h=1),
                f2v[:, :, :H // 8], f2v[:, :, H // 8:], op=ALU.add)
            hslice = h_acc[:, t * CH:(t + 1) * CH]
            if cfg.add_residual:
                nc.gpsimd.tensor_tensor(
                    t1[:], t1[:], res_sb[:, t * cfg.RESC:(t + 1) * cfg.RESC],
                    op=ALU.add)
            if cfg.relu:
                nc.gpsimd.tensor_tensor(t1[:], t1[:], bbc_sb[:], op=ALU.add)
                nc.scalar.activation(hslice, t1[:], AF.Relu)
            else:
                nc.gpsimd.tensor_tensor(hslice, t1[:], bbc_sb[:], op=ALU.add)

        # ---- interleaved emission: chunks lead their tile groups by 2 ----
        emit_chunks(0)
        if NSUB > 1:
            emit_chunks(1)
        prev = None
        for t in range(NT):
            g = bisect.bisect_right(cfg.TB, t) - 1
            if t == cfg.TB[g] and g + 2 < NSUB:
                emit_chunks(g + 2)
            st = pass1(t)
            if prev is not None:
                pass2(prev)
            prev = st
        pass2(prev)

        # ---- final stores ----
        nc.sync.dma_start(
            out=out_t[:, :].rearrange("(t p) c -> p t c", p=P),
            in_=h_acc[:].rearrange("p (t c) -> p t c", t=NT),
        )
        if cfg.calc_residual:
            nc.sync.dma_start(
                out=resout_t[:, :].rearrange("(t p) c -> p t c", p=P),
                in_=res_acc[:].rearrange("p (t c) -> p t c", t=NT),
            )
    return nc


# ---------------------------------------------------------------------------
# Host-side preprocessing
# ---------------------------------------------------------------------------

def pack_nodes(edge_index: np.ndarray, n: int, ncores: int, nt: int):
    """Assign each node to (core, tile, slot) balancing per-tile edge load.

    Returns (node_core, node_tile, node_slot, per-tile edge caps honored NB).
    """
    import heapq
    deg = np.bincount(edge_index[1], minlength=n).astype(np.int64) + 1
    nbins = ncores * nt
    for NB in (9, 10, 11):
        cap = NB * P
        order = np.argsort(-deg, kind="stable")
        loads = np.zeros(nbins, np.int64)
        slots = np.zeros(nbins, np.int32)
        node_bin = np.full(n, -1, np.int32)
        heap = [(0, b) for b in range(nbins)]
        heapq.heapify(heap)
        ok = True
        skipped = []
        for node in order:
            d = int(deg[node])
            tried = []
            placed = False
            while heap:
                load, b = heapq.heappop(heap)
                if load != loads[b] or slots[b] >= P:
                    continue   # stale or full
                if load + d <= cap:
                    node_bin[node] = b
                    loads[b] += d
                    slots[b] += 1
                    if slots[b] < P:
                        heapq.heappush(heap, (loads[b], b))
                    placed = True
                    break
                tried.append((load, b))
            for item in tried:
                heapq.heappush(heap, item)
            if not placed:
                ok = False
                break
        if ok:
            bins = node_bin
            node_core = bins // nt
            node_tile = bins % nt
            node_slot = np.zeros(n, np.int32)
            for b in range(nbins):
                idx = np.where(bins == b)[0]
                node_slot[idx] = np.arange(len(idx), dtype=np.int32)
            return node_core.astype(np.int32), node_tile.astype(np.int32), \
                node_slot, NB
    raise RuntimeError("packing failed")


def preprocess_edges(edge_index: np.ndarray, n: int, ncores: int,
                     nsub: int = 4, int16_cap: int = 32000):
    """Shard edges by packed dst; build per-core idx16 / oh streams.

    Returns (metas, layout). metas[c]:
      idx16  [128, NT*NB*8]  int16
      ohall  [128, NT*2*NB*128]  float32 (cast to bf16 later)
      usrc   [nsub] unique source-node arrays
      nodes  [NT*128] int64 node id per (tile,slot), -1 for empty
    """
    nloc = -(-n // ncores)
    NT = -(-nloc // P)
    node_core, node_tile, node_slot, NB = pack_nodes(edge_index, n, ncores, NT)
    TB = tuple(round(i * NT / nsub) for i in range(nsub + 1))
    NIDX = NB * P
    IDXW = NIDX // 16

    loops = np.arange(n, dtype=np.int64)
    src = np.concatenate([edge_index[0].astype(np.int64), loops])
    dst = np.concatenate([edge_index[1].astype(np.int64), loops])
    ecore = node_core[dst]
    etile = node_tile[dst]
    eslot = node_slot[dst]

    metas = []
    rowmax = [0] * nsub
    for c in range(ncores):
        sel = ecore == c
        s_c, t_c, d_c = src[sel], etile[sel], eslot[sel]
        order = np.argsort(t_c, kind="stable")
        s_c, t_c, d_c = s_c[order], t_c[order], d_c[order]
        tcnt = np.bincount(t_c, minlength=NT)
        tstart = np.concatenate([[0], np.cumsum(tcnt)])

        idxflat = np.zeros((NT, NIDX), np.int64)
        oh = np.zeros((NT, NB, P, P), np.float32)      # [t, b, e, d]
        usrcs = []
        for sub in range(nsub):
            tlo, thi = TB[sub], TB[sub + 1]
            allsrc = s_c[tstart[tlo]:tstart[thi]]
            usrc, inv = np.unique(allsrc, return_inverse=True)
            assert len(usrc) < int16_cap, f"int16 cap exceeded: {len(usrc)}"
            usrcs.append(usrc)
            comp = np.zeros(len(s_c), np.int64)
            comp[tstart[tlo]:tstart[thi]] = inv
            for t in range(tlo, thi):
                ne = tcnt[t]
                e0 = tstart[t]
                idxflat[t, :ne] = comp[e0:e0 + ne]
                eix = np.arange(ne)
                oh[t, eix // P, eix % P, d_c[e0:e0 + ne]] = 1.0
        ohT = np.transpose(oh, (0, 1, 3, 2))           # [t, b, d, e]
        # per tile: [ohT blocks | oh blocks] -> [t, 2, NB, P(row), P(col)]
        ohcat = np.stack([ohT, oh], axis=1)            # [t, 2, b, row, col]
        ohall = np.ascontiguousarray(
            np.transpose(ohcat, (3, 0, 1, 2, 4)).reshape(P, NT * 2 * NB * P))

        w = idxflat.reshape(NT, IDXW, 16).transpose(0, 2, 1)
        idx16 = np.tile(w, (1, 8, 1)).transpose(1, 0, 2).reshape(P, NT * IDXW)

        nodes = np.full(NT * P, -1, np.int64)
        nsel = np.where(node_core == c)[0]
        nodes[node_tile[nsel] * P + node_slot[nsel]] = nsel
        metas.append(dict(idx16=idx16.astype(np.int16), ohall=ohall,
                          usrc=usrcs, nodes=nodes))
        for sub in range(nsub):
            rowmax[sub] = max(rowmax[sub], len(metas[c]['usrc'][sub]))
    rowpad = tuple(max(P, -(-r // P) * P) for r in rowmax)
    layout = dict(NT=NT, NB=NB, TB=TB, nloc_pad=NT * P, ROWPADS=rowpad)
    return metas, layout


# ---------------------------------------------------------------------------
# Top-level kernel entry: full inputs -> full output, 8 NeuronCores
# ---------------------------------------------------------------------------
import ml_dtypes

_BF16NP = ml_dtypes.bfloat16
N_NODES = 50000
F_IN = 128
N_HEADS = 8
C_HID = 64
K_OUT = 32
NCORES = 8

_compiled_cache = {}


TUNE1 = dict(KB=4, att_pool=0, b_pool=0, cpsplit=(0, 1, 1))
TUNE2 = dict(KB=6, att_pool=0, b_pool=0, cpsplit=(0,))


def layer_cfgs(lay):
    NB = lay['NB']
    common = dict(NT=lay['NT'], NB=NB, TB=tuple(lay['TB']),
                  ROWPADS=tuple(lay['ROWPADS']))
    cfg1 = LayerCfg(Fin=F_IN, H=N_HEADS, CH=C_HID, relu=True,
                    calc_residual=True, add_residual=False,
                    **TUNE1, **common)
    cfg2 = LayerCfg(Fin=C_HID, H=N_HEADS, CH=K_OUT, relu=False,
                    calc_residual=False, add_residual=True,
                    **TUNE2, **common)
    return cfg1, cfg2


def _attT_const(att_cm, H):
    """[P, NBLK*H]: attT[p, k*H+h] = att_cm[k*128+p] iff (k*128+p)%H==h."""
    HC = len(att_cm)
    NBLK = HC // P
    out = np.zeros((P, NBLK * H), np.float32)
    j = np.arange(HC)
    out[j % P, (j // P) * H + (j % H)] = att_cm
    return out.astype(_BF16NP)


def _build_programs(lay):
    key = (lay['NT'], lay['NB'], tuple(lay['TB']), tuple(lay['ROWPADS']))
    if key in _compiled_cache:
        return _compiled_cache[key]
    cfg1, cfg2 = layer_cfgs(lay)
    ncs = []
    for cfg in (cfg1, cfg2):
        nc = bacc.Bacc("TRN2", target_bir_lowering=False, debug=False,
                       num_devices=NCORES)
        build_layer(nc, cfg)
        nc.compile()
        ncs.append(nc)
    _compiled_cache[key] = tuple(ncs)
    return _compiled_cache[key]


def _ch_major_cols(H, C):
    """col j holds original column h*C+c with j = c*H + h."""
    j = np.arange(H * C)
    c, h = j // H, j % H
    return h * C + c


def _compact_tables(xfull, m, Fin, lay):
    outs = []
    for s, rp in enumerate(lay['ROWPADS']):
        xc = np.zeros((rp, Fin), np.float32)
        u = m['usrc'][s]
        xc[:len(u)] = xfull[u]
        outs.append(np.ascontiguousarray(xc.T).astype(_BF16NP))
    return outs


def _compact_rm(xfull, m, lay):
    """Row-major compact tables, feature dim zero-padded to 128."""
    outs = []
    for s, rp in enumerate(lay['ROWPADS']):
        xc = np.zeros((rp, P), np.float32)
        u = m['usrc'][s]
        xc[:len(u), :xfull.shape[1]] = xfull[u]
        outs.append(xc.astype(_BF16NP))
    return outs


def _pad_rows(w, rows=P):
    out = np.zeros((rows, w.shape[1]), np.float32)
    out[:w.shape[0]] = w
    return out


def _local_table(xfull, Fin, m):
    nodes = m['nodes']
    xl = np.zeros((len(nodes), Fin), np.float32)
    valid = nodes >= 0
    xl[valid] = xfull[nodes[valid]]
    return np.ascontiguousarray(xl.T).astype(_BF16NP)


def kernel(x, edge_index, xyz, lin1_w, lin1_b, wl1, wr1, att1, b1,
           wl2, wr2, att2, b2):
    from concourse.bass_utils import run_bass_kernel_spmd

    x = np.asarray(x, dtype=np.float32)
    edge_index = np.asarray(edge_index)
    metas, lay = preprocess_edges(edge_index, N_NODES, NCORES)
    nc1, nc2 = _build_programs(lay)

    perm1 = _ch_major_cols(N_HEADS, C_HID)
    perm2 = _ch_major_cols(N_HEADS, K_OUT)
    wl1_cm = np.asarray(wl1, np.float32)[:, perm1]
    wr1_cm = np.asarray(wr1, np.float32)[:, perm1]
    att1_cm = np.asarray(att1, np.float32).reshape(-1)[perm1]
    wl2_cm = np.asarray(wl2, np.float32)[:, perm2]
    wr2_cm = np.asarray(wr2, np.float32)[:, perm2]
    att2_cm = np.asarray(att2, np.float32).reshape(-1)[perm2]

    attb1 = np.tile(att1_cm.reshape(1, -1), (P, 2)).astype(_BF16NP)
    attb2 = np.tile(att2_cm.reshape(1, -1), (P, 2)).astype(_BF16NP)
    b1bc = np.tile(np.asarray(b1, np.float32).reshape(1, -1), (P, 1)).astype(_BF16NP)
    b2bc = np.tile(np.asarray(b2, np.float32).reshape(1, -1), (P, 1)).astype(_BF16NP)
    linbbc = np.tile(np.asarray(lin1_b, np.float32).reshape(1, -1),
                     (P, 1)).astype(_BF16NP)

    cfg1, cfg2 = layer_cfgs(lay)
    attT1 = _attT_const(att1_cm, N_HEADS)
    attT2 = _attT_const(att2_cm, N_HEADS)

    in_maps1 = []
    for c in range(NCORES):
        m = metas[c]
        xtc = _compact_tables(x, m, F_IN, lay)
        im = dict(
            xTloc=_local_table(x, F_IN, m),
            wl=_pad_rows(wl1_cm).astype(_BF16NP), wr=wr1_cm.astype(_BF16NP),
            attb=attb1, bbc=b1bc, idx16=m['idx16'],
            ohall=m['ohall'].astype(_BF16NP),
            linw=np.asarray(lin1_w, np.float32).astype(_BF16NP), linb=linbbc)
        for i, tab in enumerate(xtc):
            im[f'xTc{i}'] = tab
        if cfg1.KB:
            im['attT'] = attT1
            for i, tab in enumerate(_compact_rm(x, m, lay)):
                im[f'xcm{i}'] = tab
        in_maps1.append(im)
    res1 = run_bass_kernel_spmd(nc1, in_maps1, core_ids=list(range(NCORES)))

    h_full = np.zeros((N_NODES, C_HID), np.float32)
    res_full = np.zeros((N_NODES, K_OUT), np.float32)
    for c in range(NCORES):
        nodes = metas[c]['nodes']
        valid = nodes >= 0
        h_full[nodes[valid]] = np.asarray(
            res1.results[c]["out"], np.float32)[valid]
        res_full[nodes[valid]] = np.asarray(
            res1.results[c]["resout"], np.float32)[valid]

    in_maps2 = []
    for c in range(NCORES):
        m = metas[c]
        htc = _compact_tables(h_full, m, C_HID, lay)
        resin = np.zeros((lay['nloc_pad'], K_OUT), np.float32)
        nodes = m['nodes']
        valid = nodes >= 0
        resin[valid] = res_full[nodes[valid]]
        im = dict(
            xTloc=_local_table(h_full, C_HID, m),
            wl=_pad_rows(wl2_cm).astype(_BF16NP), wr=wr2_cm.astype(_BF16NP),
            attb=attb2, bbc=b2bc, idx16=m['idx16'],
            ohall=m['ohall'].astype(_BF16NP),
            resin=resin.astype(_BF16NP))
        for i, tab in enumerate(htc):
            im[f'xTc{i}'] = tab
        if cfg2.KB:
            im['attT'] = attT2
            for i, tab in enumerate(_compact_rm(h_full, m, lay)):
                im[f'xcm{i}'] = tab
        in_maps2.append(im)
    res2 = run_bass_kernel_spmd(nc2, in_maps2, core_ids=list(range(NCORES)))

    out = np.zeros((N_NODES, K_OUT), np.float32)
    for c in range(NCORES):
        nodes = metas[c]['nodes']
        valid = nodes >= 0
        out[nodes[valid]] = np.asarray(
            res2.results[c]["out"], np.float32)[valid]
    return out

